# revision 8
# baseline (speedup 1.0000x reference)
"""Bass/Trainium2 kernel for DeathRxnLayer (scatter_memory).

reference:
    muTE   = zeros_like(mu);   muTE[:, i_sp] = -mu[:, i_sp]
    row    = -ncov[:, i_sp, :];  row[:, i_sp] = -2*ncov[:, i_sp, i_sp] + mu[:, i_sp]
    ncovTE = zeros_like(ncov); ncovTE[:, i_sp, :] = row; ncovTE[:, :, i_sp] = row

Strategy: pure data parallelism over the batch dim (1024 samples per core).
The output is ~516 MB of mostly zeros, so the kernel is HBM-write-bound.
Per core, 128 samples are assembled per chunk in an SBUF tile laid out
sample-per-partition ([128 part, 16384 f32] = one full 128x128 matrix in each
partition's free dim). The zero background is memset once into two persistent
buffers; every iteration only overwrites row i_sp (contiguous 512B) and column
i_sp (stride-512B) slots -- their addresses never change -- then one contiguous
8MB DMA stores the chunk. Big stores alternate between the two HWDGE rings
(SP / ACT) so transfers overlap. Raw bass with explicit semaphores (Tile's
scheduler emits >1 sem-wait on single DVE instructions for this dependency
pattern, which walrus codegen rejects; standalone wait_ge is fine).

The negated row is computed directly in the big tile's row slot; the diagonal
cell doubles as the column's diagonal element, and the column above/below the
diagonal is copied from the row slot (symmetry), so no intermediate row buffer
or overlapping same-engine writes exist.

`reps` > 1 repeats the whole chunk pipeline inside one NEFF (writing the same
outputs again) -- used only for differential wall-clock timing, since this
environment has no NTFF profiling hook.
"""

import numpy as np

import concourse.bass as bass
import concourse.mybir as mybir
from concourse.bass_utils import run_bass_kernel_spmd

B, N = 8192, 128
N_CORES = 8
BS = B // N_CORES  # samples per core
C = 128            # samples per chunk (one per SBUF partition)
T = BS // C        # chunks per core
F32 = mybir.dt.float32

_nc_cache: dict[tuple[int, int], bass.Bass] = {}


def _build(i_sp: int, reps: int = 1) -> bass.Bass:
    nc = bass.Bass()
    mu = nc.dram_tensor("mu", [BS, N], F32, kind="ExternalInput")
    ncov = nc.dram_tensor("ncov", [BS, N, N], F32, kind="ExternalInput")
    muTE = nc.dram_tensor("muTE", [BS, N], F32, kind="ExternalOutput")
    ncovTE = nc.dram_tensor("ncovTE", [BS, N * N], F32, kind="ExternalOutput")

    G = T * reps  # total chunk iterations (chunk index = g % T)
    half = (N * N) // 2
    with (
        nc.sbuf_tensor([C, N * N], F32) as big0,
        nc.sbuf_tensor([C, N * N], F32) as big1,
        nc.sbuf_tensor([C, T * N], F32) as muT,
        nc.sbuf_tensor([C, 3 * N], F32) as srcb,
        nc.sbuf_tensor([C, 3 * N], F32) as mucb,
        nc.semaphore("sem_in") as sem_in,   # input loads (2 DMAs x 16 per iter)
        nc.semaphore("sem_be") as sem_be,   # even big stores (SP ring)
        nc.semaphore("sem_bo") as sem_bo,   # odd big stores + muTE (ACT ring)
        nc.semaphore("sem_v") as sem_v,     # muT column completions (rep 0)
        nc.semaphore("sem_p") as sem_p,     # gpsimd one-time memsets
        nc.semaphore("sem_d") as sem_d,     # DVE retirement counter
        nc.Block() as block,
    ):
        bigs = [big0, big1]
        dvals = [0] * G  # sem_d value once iteration g's big-tile ops retired

        @block.gpsimd
        def _(g_eng):
            g_eng.memset(big0[:, half:], 0.0).then_inc(sem_p, 1)
            g_eng.memset(big1[:, half:], 0.0).then_inc(sem_p, 1)

        @block.vector
        def _(v):
            d = 0  # sem_d value after each tracked DVE op

            def inc(ins):
                nonlocal d
                d += 1
                ins.then_inc(sem_d, 1)

            inc(v.memset(big0[:, :half], 0.0))
            inc(v.memset(big1[:, :half], 0.0))
            inc(v.memset(muT[:], 0.0))
            prev_tail = [0, 0]  # dvals of last iteration that used this buffer
            for g in range(G):
                t = g % T
                b = bigs[g % 2]
                r = g % 3
                s = srcb[:, r * N : (r + 1) * N]
                mc = mucb[:, r * N : (r + 1) * N]
                brow = b[:, i_sp * N : (i_sp + 1) * N]
                b3 = b[:].rearrange("c (p j) -> c p j", j=N)

                v.wait_ge(sem_in, 32 * (g + 1))
                if g < 2:
                    v.wait_ge(sem_p, g + 1)  # gpsimd half-memset of big[g]
                    v.wait_ge(sem_d, 3)      # DVE memsets retired
                else:
                    if g % 2 == 0:
                        v.wait_ge(sem_be, 16 * (g // 2))        # store g-2 done
                    else:
                        v.wait_ge(sem_bo, 16 * ((g - 1) // 2))  # store g-2 done
                    v.wait_ge(sem_d, prev_tail[g % 2])  # iter g-2 DVE retired

                # row slot <- -src
                inc(v.tensor_scalar_mul(brow, s, -1.0))
                v.wait_ge(sem_d, d)  # row slot fully written before deriving
                # diag cell (serves as both row and column diagonal element)
                inc(
                    v.tensor_scalar(
                        brow[:, i_sp : i_sp + 1],
                        s[:, i_sp : i_sp + 1],
                        -2.0,
                        mc[:, i_sp : i_sp + 1],
                        mybir.AluOpType.mult,
                        mybir.AluOpType.add,
                    )
                )
                # column i_sp above/below the diagonal, copied from the row slot
                if i_sp > 0:
                    inc(v.tensor_copy(b3[:, :i_sp, i_sp], brow[:, :i_sp]))
                if i_sp < N - 1:
                    inc(v.tensor_copy(b3[:, i_sp + 1 :, i_sp], brow[:, i_sp + 1 :]))
                prev_tail[g % 2] = d
                dvals[g] = d
                if g < T:
                    # muTE column for this chunk (written once, in rep 0)
                    col = t * N + i_sp
                    v.tensor_scalar_mul(
                        muT[:, col : col + 1], mc[:, i_sp : i_sp + 1], -1.0
                    ).then_inc(sem_v, 1)

        @block.sync
        def _(sp):
            for g in range(G):
                t = g % T
                r = g % 3
                if g >= 1:
                    # separates each iteration's loads so every 32-step of
                    # sem_in is a valid wait point; also guards the input
                    # ring WAR (slot g%3 free once DVE finished iter g-3)
                    sp.wait_ge(sem_d, dvals[g - 1])
                sl = slice(t * C, (t + 1) * C)
                sp.dma_start(
                    out=srcb[:, r * N : (r + 1) * N], in_=ncov[sl, i_sp, :]
                ).then_inc(sem_in, 16)
                sp.dma_start(
                    out=mucb[:, r * N : (r + 1) * N], in_=mu[sl, :]
                ).then_inc(sem_in, 16)
                if g % 2 == 0:
                    sp.wait_ge(sem_d, dvals[g])  # chunk staged
                    sp.dma_start(out=ncovTE[sl, :], in_=bigs[0][:]).then_inc(
                        sem_be, 16
                    )
            # all output DMAs landed before the kernel ends
            sp.wait_ge(sem_be, 16 * ((G + 1) // 2))
            sp.wait_ge(sem_bo, 16 * (G // 2 + 1))

        @block.scalar
        def _(act):
            for g in range(1, G, 2):
                t = g % T
                act.wait_ge(sem_d, dvals[g])  # chunk staged
                sl = slice(t * C, (t + 1) * C)
                act.dma_start(out=ncovTE[sl, :], in_=bigs[1][:]).then_inc(sem_bo, 16)
            act.wait_ge(sem_v, T)  # all muT columns written
            act.dma_start(
                out=muTE[:].rearrange("(t c) j -> c t j", c=C),
                in_=muT[:].rearrange("c (t j) -> c t j", j=N),
            ).then_inc(sem_bo, 16)

    return nc


def _get_nc(i_sp: int, reps: int = 1) -> bass.Bass:
    key = (i_sp, reps)
    if key not in _nc_cache:
        _nc_cache[key] = _build(i_sp, reps)
    return _nc_cache[key]


def run_shards(mu, ncov, i_sp: int, reps: int = 1):
    nc = _get_nc(i_sp, reps)
    in_maps = [
        {"mu": mu[c * BS : (c + 1) * BS], "ncov": ncov[c * BS : (c + 1) * BS]}
        for c in range(N_CORES)
    ]
    return run_bass_kernel_spmd(nc, in_maps, list(range(N_CORES))).results


def kernel(mu, ncov, i_sp):
    mu = np.ascontiguousarray(np.asarray(mu, dtype=np.float32))
    ncov = np.ascontiguousarray(np.asarray(ncov, dtype=np.float32))
    res = run_shards(mu, ncov, int(np.asarray(i_sp)))
    muTE = np.concatenate([r["muTE"] for r in res], axis=0)
    ncovTE = np.concatenate([r["ncovTE"] for r in res], axis=0).reshape(B, N, N)
    return muTE, ncovTE


# revision 10
# speedup vs baseline: 1.2764x; 1.2764x over previous
"""Bass/Trainium2 kernel for DeathRxnLayer (scatter_memory).

reference:
    muTE   = zeros_like(mu);   muTE[:, i_sp] = -mu[:, i_sp]
    row    = -ncov[:, i_sp, :];  row[:, i_sp] = -2*ncov[:, i_sp, i_sp] + mu[:, i_sp]
    ncovTE = zeros_like(ncov); ncovTE[:, i_sp, :] = row; ncovTE[:, :, i_sp] = row

Strategy: pure data parallelism over the batch dim (1024 samples per core).
The output is ~516 MB of mostly zeros, so the kernel is HBM-write-bound
(~64.5 MB written per core; only ~1 MB read). Per core, 128 samples are
assembled per chunk in an SBUF tile laid out sample-per-partition
([128 part, 16384 f32] = one full 128x128 matrix in each partition's free
dim). The zero background is memset once into two persistent buffers; every
iteration only overwrites row i_sp (contiguous 512B) and column i_sp
(stride-512B) slots -- their addresses never change -- then one contiguous
8MB DMA stores the chunk. Big stores alternate between the two HWDGE rings
(SP / ACT) so transfers overlap and saturate HBM write bandwidth. All input
reads happen in two bulk DMAs up front, hidden under the memset window, so
the store rings stay pure (measured: per-chunk loads in the SP ring cost
~19us/rep).

Raw bass with explicit semaphores: Tile's scheduler emits >1 sem-wait on
single DVE instructions for this dependency pattern, which walrus codegen
rejects; standalone wait_ge instructions are fine. The negated row is
computed directly in the big tile's row slot; the diagonal cell doubles as
the column's diagonal element, and the column above/below the diagonal is
copied from the row slot (symmetry), so no overlapping same-engine writes
exist without a sem edge.

`reps` > 1 repeats the store pipeline inside one NEFF (rewriting the same
outputs) -- used only for differential wall-clock timing, since this
environment has no NTFF profiling hook. `timing=True` additionally makes the
big outputs Internal DRAM scratch with a tiny token output, so timing calls
move almost no data over the axon tunnel.
"""

import numpy as np

import concourse.bass as bass
import concourse.mybir as mybir
from concourse.bass_utils import run_bass_kernel_spmd

B, N = 8192, 128
N_CORES = 8
BS = B // N_CORES  # samples per core
C = 128            # samples per chunk (one per SBUF partition)
T = BS // C        # chunks per core
F32 = mybir.dt.float32

_nc_cache: dict[tuple, bass.Bass] = {}


def _build(i_sp: int, reps: int = 1, timing: bool = False) -> bass.Bass:
    nc = bass.Bass()
    mu = nc.dram_tensor("mu", [BS, N], F32, kind="ExternalInput")
    ncov = nc.dram_tensor("ncov", [BS, N, N], F32, kind="ExternalInput")
    out_kind = {} if timing else {"kind": "ExternalOutput"}
    muTE = nc.dram_tensor("muTE", [BS, N], F32, **out_kind)
    ncovTE = nc.dram_tensor("ncovTE", [BS, N * N], F32, **out_kind)
    tok = nc.dram_tensor("tok", [1, 1], F32, kind="ExternalOutput") if timing else None

    G = T * reps  # total chunk iterations (chunk index = g % T)
    half = (N * N) // 2
    with (
        nc.sbuf_tensor([C, N * N], F32) as big0,
        nc.sbuf_tensor([C, N * N], F32) as big1,
        nc.sbuf_tensor([C, T * N], F32) as muT,
        nc.sbuf_tensor([C, T * N], F32) as srcb,
        nc.sbuf_tensor([C, T * N], F32) as mucb,
        nc.semaphore("sem_in") as sem_in,   # the two bulk input loads
        nc.semaphore("sem_be") as sem_be,   # even big stores (SP ring)
        nc.semaphore("sem_bo") as sem_bo,   # odd big stores + muTE (ACT ring)
        nc.semaphore("sem_v") as sem_v,     # muT column completions (rep 0)
        nc.semaphore("sem_p") as sem_p,     # gpsimd one-time memsets
        nc.semaphore("sem_d") as sem_d,     # DVE retirement counter
        nc.Block() as block,
    ):
        bigs = [big0, big1]
        dvals = [0] * G  # sem_d value once iteration g's big-tile ops retired

        @block.gpsimd
        def _(g_eng):
            g_eng.memset(big0[:, half:], 0.0).then_inc(sem_p, 1)
            g_eng.memset(big1[:, half:], 0.0).then_inc(sem_p, 1)

        @block.vector
        def _(v):
            d = 0  # sem_d value after each tracked DVE op

            def inc(ins):
                nonlocal d
                d += 1
                ins.then_inc(sem_d, 1)

            inc(v.memset(big0[:, :half], 0.0))
            inc(v.memset(big1[:, :half], 0.0))
            inc(v.memset(muT[:], 0.0))
            prev_tail = [0, 0]  # dvals of last iteration that used this buffer
            v.wait_ge(sem_in, 32)  # both bulk loads complete
            for g in range(G):
                t = g % T
                b = bigs[g % 2]
                s = srcb[:, t * N : (t + 1) * N]
                mc = mucb[:, t * N : (t + 1) * N]
                brow = b[:, i_sp * N : (i_sp + 1) * N]
                b3 = b[:].rearrange("c (p j) -> c p j", j=N)

                if g < 2:
                    v.wait_ge(sem_p, g + 1)  # gpsimd half-memset of big[g]
                    v.wait_ge(sem_d, 3)      # DVE memsets retired
                else:
                    if g % 2 == 0:
                        v.wait_ge(sem_be, 16 * (g // 2))        # store g-2 done
                    else:
                        v.wait_ge(sem_bo, 16 * ((g - 1) // 2))  # store g-2 done
                    v.wait_ge(sem_d, prev_tail[g % 2])  # iter g-2 DVE retired

                # row slot <- -src
                inc(v.tensor_scalar_mul(brow, s, -1.0))
                v.wait_ge(sem_d, d)  # row slot fully written before deriving
                # diag cell (serves as both row and column diagonal element)
                inc(
                    v.tensor_scalar(
                        brow[:, i_sp : i_sp + 1],
                        s[:, i_sp : i_sp + 1],
                        -2.0,
                        mc[:, i_sp : i_sp + 1],
                        mybir.AluOpType.mult,
                        mybir.AluOpType.add,
                    )
                )
                # column i_sp above/below the diagonal, copied from the row slot
                if i_sp > 0:
                    inc(v.tensor_copy(b3[:, :i_sp, i_sp], brow[:, :i_sp]))
                if i_sp < N - 1:
                    inc(v.tensor_copy(b3[:, i_sp + 1 :, i_sp], brow[:, i_sp + 1 :]))
                prev_tail[g % 2] = d
                dvals[g] = d
                if g < T:
                    # muTE column for this chunk (written once, in rep 0)
                    col = t * N + i_sp
                    v.tensor_scalar_mul(
                        muT[:, col : col + 1], mc[:, i_sp : i_sp + 1], -1.0
                    ).then_inc(sem_v, 1)

        @block.sync
        def _(sp):
            # bulk input loads: all T chunks of ncov[:, i_sp, :] and mu
            sp.dma_start(
                out=srcb[:].rearrange("c (t j) -> c t j", j=N),
                in_=ncov[:].rearrange("(t c) p j -> c t p j", c=C)[:, :, i_sp, :],
            ).then_inc(sem_in, 16)
            sp.dma_start(
                out=mucb[:].rearrange("c (t j) -> c t j", j=N),
                in_=mu[:].rearrange("(t c) j -> c t j", c=C),
            ).then_inc(sem_in, 16)
            for g in range(0, G, 2):
                t = g % T
                sl = slice(t * C, (t + 1) * C)
                sp.wait_ge(sem_d, dvals[g])  # chunk staged
                sp.dma_start(out=ncovTE[sl, :], in_=bigs[0][:]).then_inc(sem_be, 16)
            # all output DMAs landed before the kernel ends
            sp.wait_ge(sem_be, 16 * ((G + 1) // 2))
            sp.wait_ge(sem_bo, 16 * (G // 2 + 1))
            if tok is not None:
                sp.dma_start(out=tok[:], in_=big0[0:1, 0:1]).then_inc(sem_be, 16)
                sp.wait_ge(sem_be, 16 * ((G + 1) // 2 + 1))

        @block.scalar
        def _(act):
            for g in range(1, G, 2):
                t = g % T
                sl = slice(t * C, (t + 1) * C)
                act.wait_ge(sem_d, dvals[g])  # chunk staged
                act.dma_start(out=ncovTE[sl, :], in_=bigs[1][:]).then_inc(sem_bo, 16)
            act.wait_ge(sem_v, T)  # all muT columns written
            act.dma_start(
                out=muTE[:].rearrange("(t c) j -> c t j", c=C),
                in_=muT[:].rearrange("c (t j) -> c t j", j=N),
            ).then_inc(sem_bo, 16)

    return nc


def _get_nc(i_sp: int, reps: int = 1, timing: bool = False) -> bass.Bass:
    key = (i_sp, reps, timing)
    if key not in _nc_cache:
        _nc_cache[key] = _build(i_sp, reps, timing)
    return _nc_cache[key]


def run_shards(mu, ncov, i_sp: int, reps: int = 1):
    nc = _get_nc(i_sp, reps)
    in_maps = [
        {"mu": mu[c * BS : (c + 1) * BS], "ncov": ncov[c * BS : (c + 1) * BS]}
        for c in range(N_CORES)
    ]
    return run_bass_kernel_spmd(nc, in_maps, list(range(N_CORES))).results


def kernel(mu, ncov, i_sp):
    mu = np.ascontiguousarray(np.asarray(mu, dtype=np.float32))
    ncov = np.ascontiguousarray(np.asarray(ncov, dtype=np.float32))
    res = run_shards(mu, ncov, int(np.asarray(i_sp)))
    muTE = np.concatenate([r["muTE"] for r in res], axis=0)
    ncovTE = np.concatenate([r["ncovTE"] for r in res], axis=0).reshape(B, N, N)
    return muTE, ncovTE


# revision 11
# speedup vs baseline: 1.4948x; 1.1711x over previous
"""Bass/Trainium2 kernel for DeathRxnLayer (scatter_memory).

reference:
    muTE   = zeros_like(mu);   muTE[:, i_sp] = -mu[:, i_sp]
    row    = -ncov[:, i_sp, :];  row[:, i_sp] = -2*ncov[:, i_sp, i_sp] + mu[:, i_sp]
    ncovTE = zeros_like(ncov); ncovTE[:, i_sp, :] = row; ncovTE[:, :, i_sp] = row

Strategy: pure data parallelism over the batch dim (1024 samples per core).
The output is ~516 MB of mostly zeros, so the kernel is HBM-write-bound
(~64.5 MB written per core; only ~1 MB read). Per core, 128 samples are
assembled per chunk in an SBUF tile laid out sample-per-partition
([128 part, 16384 f32] = one full 128x128 matrix in each partition's free
dim). The zero background is memset once into two persistent buffers; every
iteration only overwrites row i_sp (contiguous 512B) and column i_sp
(stride-512B) slots -- their addresses never change -- then one contiguous
8MB DMA stores the chunk. Big stores alternate between the two HWDGE rings
(SP / ACT) so transfers overlap and saturate HBM write bandwidth. All input
reads happen in two bulk DMAs up front, hidden under the memset window, so
the store rings stay pure (measured: per-chunk loads in the SP ring cost
~19us/rep).

Raw bass with explicit semaphores: Tile's scheduler emits >1 sem-wait on
single DVE instructions for this dependency pattern, which walrus codegen
rejects; standalone wait_ge instructions are fine. The negated row is
computed directly in the big tile's row slot; the diagonal cell doubles as
the column's diagonal element, and the column above/below the diagonal is
copied from the row slot (symmetry), so no overlapping same-engine writes
exist without a sem edge.

`reps` > 1 repeats the store pipeline inside one NEFF (rewriting the same
outputs) -- used only for differential wall-clock timing, since this
environment has no NTFF profiling hook. `timing=True` additionally makes the
big outputs Internal DRAM scratch with a tiny token output, so timing calls
move almost no data over the axon tunnel.
"""

import numpy as np

import concourse.bass as bass
import concourse.mybir as mybir
from concourse.bass_utils import run_bass_kernel_spmd

B, N = 8192, 128
N_CORES = 8
BS = B // N_CORES  # samples per core
C = 128            # samples per chunk (one per SBUF partition)
T = BS // C        # chunks per core
F32 = mybir.dt.float32

_nc_cache: dict[tuple, bass.Bass] = {}


def _build(i_sp: int, reps: int = 1, timing: bool = False) -> bass.Bass:
    nc = bass.Bass()
    mu = nc.dram_tensor("mu", [BS, N], F32, kind="ExternalInput")
    ncov = nc.dram_tensor("ncov", [BS, N, N], F32, kind="ExternalInput")
    out_kind = {} if timing else {"kind": "ExternalOutput"}
    muTE = nc.dram_tensor("muTE", [BS, N], F32, **out_kind)
    ncovTE = nc.dram_tensor("ncovTE", [BS, N * N], F32, **out_kind)
    tok = nc.dram_tensor("tok", [1, 1], F32, kind="ExternalOutput") if timing else None

    G = T * reps  # total chunk iterations (chunk index = g % T)
    half = (N * N) // 2
    with (
        nc.sbuf_tensor([C, N * N], F32) as big0,
        nc.sbuf_tensor([C, N * N], F32) as big1,
        nc.sbuf_tensor([C, T * N], F32) as muT,
        nc.sbuf_tensor([C, T * N], F32) as srcb,
        nc.sbuf_tensor([C, T * N], F32) as mucb,
        nc.semaphore("sem_in") as sem_in,   # the two bulk input loads
        nc.semaphore("sem_be") as sem_be,   # even big stores (SP ring)
        nc.semaphore("sem_bo") as sem_bo,   # odd big stores + muTE (ACT ring)
        nc.semaphore("sem_v") as sem_v,     # muT column completions (rep 0)
        nc.semaphore("sem_p") as sem_p,     # gpsimd one-time memsets
        nc.semaphore("sem_d") as sem_d,     # DVE retirement counter
        nc.Block() as block,
    ):
        bigs = [big0, big1]
        dvals = [0] * G  # sem_d value once iteration g's big-tile ops retired

        @block.gpsimd
        def _(g_eng):
            g_eng.memset(big0[:, half:], 0.0).then_inc(sem_p, 1)
            g_eng.memset(big1[:, half:], 0.0).then_inc(sem_p, 1)

        @block.vector
        def _(v):
            d = 0  # sem_d value after each tracked DVE op

            def inc(ins):
                nonlocal d
                d += 1
                ins.then_inc(sem_d, 1)

            inc(v.memset(big0[:, :half], 0.0))
            inc(v.memset(big1[:, :half], 0.0))
            inc(v.memset(muT[:], 0.0))
            prev_tail = [0, 0]  # dvals of last iteration that used this buffer
            v.wait_ge(sem_in, 32)  # both bulk loads complete
            for g in range(G):
                t = g % T
                b = bigs[g % 2]
                s = srcb[:, t * N : (t + 1) * N]
                mc = mucb[:, t * N : (t + 1) * N]
                brow = b[:, i_sp * N : (i_sp + 1) * N]
                b3 = b[:].rearrange("c (p j) -> c p j", j=N)

                if g < 2:
                    v.wait_ge(sem_p, g + 1)  # gpsimd half-memset of big[g]
                    v.wait_ge(sem_d, 3)      # DVE memsets retired
                else:
                    if g % 2 == 0:
                        v.wait_ge(sem_be, 16 * (g // 2))        # store g-2 done
                    else:
                        v.wait_ge(sem_bo, 16 * ((g - 1) // 2))  # store g-2 done
                    v.wait_ge(sem_d, prev_tail[g % 2])  # iter g-2 DVE retired

                # row slot <- -src
                inc(v.tensor_scalar_mul(brow, s, -1.0))
                v.wait_ge(sem_d, d)  # row slot fully written before deriving
                # diag cell (serves as both row and column diagonal element)
                inc(
                    v.tensor_scalar(
                        brow[:, i_sp : i_sp + 1],
                        s[:, i_sp : i_sp + 1],
                        -2.0,
                        mc[:, i_sp : i_sp + 1],
                        mybir.AluOpType.mult,
                        mybir.AluOpType.add,
                    )
                )
                # column i_sp above/below the diagonal, copied from the row slot
                if i_sp > 0:
                    inc(v.tensor_copy(b3[:, :i_sp, i_sp], brow[:, :i_sp]))
                if i_sp < N - 1:
                    inc(v.tensor_copy(b3[:, i_sp + 1 :, i_sp], brow[:, i_sp + 1 :]))
                prev_tail[g % 2] = d
                dvals[g] = d
                if g < T:
                    # muTE column for this chunk (written once, in rep 0)
                    col = t * N + i_sp
                    v.tensor_scalar_mul(
                        muT[:, col : col + 1], mc[:, i_sp : i_sp + 1], -1.0
                    ).then_inc(sem_v, 1)

        @block.sync
        def _(sp):
            # bulk input loads: all T chunks of ncov[:, i_sp, :] and mu
            sp.dma_start(
                out=srcb[:].rearrange("c (t j) -> c t j", j=N),
                in_=ncov[:].rearrange("(t c) p j -> c t p j", c=C)[:, :, i_sp, :],
            ).then_inc(sem_in, 16)
            sp.dma_start(
                out=mucb[:].rearrange("c (t j) -> c t j", j=N),
                in_=mu[:].rearrange("(t c) j -> c t j", c=C),
            ).then_inc(sem_in, 16)
            for g in range(0, G, 2):
                t = g % T
                sl = slice(t * C, (t + 1) * C)
                sp.wait_ge(sem_d, dvals[g])  # chunk staged
                sp.dma_start(out=ncovTE[sl, :], in_=bigs[0][:]).then_inc(sem_be, 16)
            # all output DMAs landed before the kernel ends
            sp.wait_ge(sem_be, 16 * ((G + 1) // 2))
            sp.wait_ge(sem_bo, 16 * (G // 2 + 1))
            if tok is not None:
                sp.dma_start(out=tok[:], in_=big0[0:1, 0:1]).then_inc(sem_be, 16)
                sp.wait_ge(sem_be, 16 * ((G + 1) // 2 + 1))

        @block.scalar
        def _(act):
            for g in range(1, G, 2):
                t = g % T
                sl = slice(t * C, (t + 1) * C)
                act.wait_ge(sem_d, dvals[g])  # chunk staged
                act.dma_start(out=ncovTE[sl, :], in_=bigs[1][:]).then_inc(sem_bo, 16)
            act.wait_ge(sem_v, T)  # all muT columns written
            act.dma_start(
                out=muTE[:].rearrange("(t c) j -> c t j", c=C),
                in_=muT[:].rearrange("c (t j) -> c t j", j=N),
            ).then_inc(sem_bo, 16)

    return nc


def _get_nc(i_sp: int, reps: int = 1, timing: bool = False) -> bass.Bass:
    key = (i_sp, reps, timing)
    if key not in _nc_cache:
        _nc_cache[key] = _build(i_sp, reps, timing)
    return _nc_cache[key]


def run_shards(mu, ncov, i_sp: int, reps: int = 1):
    nc = _get_nc(i_sp, reps)
    in_maps = [
        {"mu": mu[c * BS : (c + 1) * BS], "ncov": ncov[c * BS : (c + 1) * BS]}
        for c in range(N_CORES)
    ]
    # the axon-tunneled device occasionally needs a recovery window right
    # after a previous session crashed; retry transient runtime failures
    last = None
    for attempt in range(3):
        try:
            return run_bass_kernel_spmd(nc, in_maps, list(range(N_CORES))).results
        except Exception as e:  # jax.errors.JaxRuntimeError and friends
            last = e
            if attempt < 2:
                import time as _time

                _time.sleep(90)
    raise last


def kernel(mu, ncov, i_sp):
    mu = np.ascontiguousarray(np.asarray(mu, dtype=np.float32))
    ncov = np.ascontiguousarray(np.asarray(ncov, dtype=np.float32))
    res = run_shards(mu, ncov, int(np.asarray(i_sp)))
    muTE = np.concatenate([r["muTE"] for r in res], axis=0)
    ncovTE = np.concatenate([r["ncovTE"] for r in res], axis=0).reshape(B, N, N)
    return muTE, ncovTE
